# revision 1
# baseline (speedup 1.0000x reference)
"""Trainium2 Bass kernel for DifferentialDualAttentionInteractiveBlock.

Self-contained: hardcodes shapes (nW=1024, N=64, DIM=192, H=6, d=32),
shards data-parallel over windows across 8 NeuronCores.
"""
import sys

sys.path.insert(0, "/opt/trn_rl_repo")

import numpy as np
import ml_dtypes

import concourse.bass as bass  # noqa: F401
import concourse.bacc as bacc
import concourse.mybir as mybir
from concourse.tile import TileContext

BF16 = ml_dtypes.bfloat16
F32 = np.float32

WS = 8
N = 64
DIM = 192
H = 6
HD = 32
SCALE = HD ** -0.5
N_CORES = 8
NW = 1024
NWC = NW // N_CORES
GROUP = 8
UNITS_PER_GROUP = GROUP // 2
N_GROUPS = NWC // GROUP
TOK_C = NWC * N

_COMPILED = {}


def _rel_pos_bias(rpb_table):
    coords = np.stack(np.meshgrid(np.arange(WS), np.arange(WS), indexing="ij"))
    cf = coords.reshape(2, -1)
    rc = (cf[:, :, None] - cf[:, None, :]).transpose(1, 2, 0).astype(np.int64)
    rc[:, :, 0] += WS - 1
    rc[:, :, 1] += WS - 1
    rc[:, :, 0] *= 2 * WS - 1
    idx = rc.sum(-1)
    bias = np.asarray(rpb_table)[idx.reshape(-1)].reshape(N, N, H)
    return bias.transpose(2, 0, 1).astype(np.float64)  # [H, q, k]


def _sigmoid_clip(lam):
    s = 1.0 / (1.0 + np.exp(-np.float64(lam)))
    return float(np.clip(s, 0.01, 0.99))


def _strip(h, br):
    """(row-strip, index-within-strip) for head h, branch br."""
    if h < 4:
        return h, 0
    return (h - 4) + 2 * br, 1


def _host_prep(inputs):
    x = np.asarray(inputs["x_windows"], dtype=F32)
    y = np.asarray(inputs["y_windows"], dtype=F32)

    qkv = np.asarray(inputs["sa_qkv_w"], dtype=np.float64)
    sa_ct = np.asarray(inputs["sa_ct_w"], dtype=np.float64)
    sa_cr = np.asarray(inputs["sa_cr_w"], dtype=np.float64)
    ca_q = np.asarray(inputs["ca_q_w"], dtype=np.float64)
    ca_kv = np.asarray(inputs["ca_kv_w"], dtype=np.float64)
    ca_ct = np.asarray(inputs["ca_ct_w"], dtype=np.float64)
    ca_cr = np.asarray(inputs["ca_cr_w"], dtype=np.float64)
    sa_enh = float(np.asarray(inputs["sa_enh"]))
    ca_enh = float(np.asarray(inputs["ca_enh"]))

    Wq_sa = qkv[0:DIM] * SCALE
    Wk_sa = qkv[DIM:2 * DIM]
    Wv_sa = qkv[2 * DIM:3 * DIM]
    Wq_ca = ca_q * SCALE
    Wk_ca = ca_kv[0:DIM]
    Wv_ca = ca_kv[DIM:2 * DIM]

    def ct(W, enh, cross):
        return np.concatenate([W, enh * (W @ cross)], axis=1)

    def cr(W, enh, cross):
        return np.concatenate([enh * (W @ cross), W], axis=1)

    W_fm_t = np.concatenate([
        ct(Wq_sa, sa_enh, sa_cr), ct(Wk_sa, sa_enh, sa_cr),
        ct(Wq_ca, ca_enh, ca_cr), ct(Wk_ca, ca_enh, ca_cr)], axis=0)
    W_fm_r = np.concatenate([
        cr(Wq_sa, sa_enh, sa_ct), cr(Wk_sa, sa_enh, sa_ct),
        cr(Wq_ca, ca_enh, ca_ct), cr(Wk_ca, ca_enh, ca_ct)], axis=0)
    # permute rows so q-head-h and k-head-h share partition offsets:
    # oc0=q_sa h0-3, oc1=k_sa h0-3, oc2=[q4s,q5s,q4c,q5c], oc3=[k4s,k5s,k4c,k5c],
    # oc4=q_ca h0-3, oc5=k_ca h0-3
    perm = np.concatenate([
        np.arange(0, 128), np.arange(192, 320),
        np.arange(128, 192), np.arange(384 + 128, 384 + 192),
        np.arange(320, 384), np.arange(576 + 128, 576 + 192),
        np.arange(384, 512), np.arange(576, 704)])
    W_fm_t = W_fm_t[perm]
    W_fm_r = W_fm_r[perm]
    wfm = np.zeros((2, 6, 3, 128, 128), dtype=BF16)
    for s, W in enumerate([W_fm_t, W_fm_r]):
        for oc in range(6):
            for kc in range(3):
                blk = W[oc * 128:(oc + 1) * 128, kc * 128:(kc + 1) * 128]
                wfm[s, oc, kc] = blk.T.astype(BF16)

    W_v_t = np.concatenate([ct(Wv_sa, sa_enh, sa_cr),
                            ct(Wv_ca, ca_enh, ca_cr)], axis=0)
    W_v_r = np.concatenate([cr(Wv_sa, sa_enh, sa_ct),
                            cr(Wv_ca, ca_enh, ca_ct)], axis=0)
    # wv[kc] : [128, 768] = [t-side (384) | r-side (384)]
    wv = np.zeros((3, 128, 768), dtype=BF16)
    for kc in range(3):
        wv[kc, :, 0:384] = W_v_t[:, kc * 128:(kc + 1) * 128].T.astype(BF16)
        wv[kc, :, 384:768] = W_v_r[:, kc * 128:(kc + 1) * 128].T.astype(BF16)

    bias = _rel_pos_bias(inputs["rpb_table"])
    # exp(bias), both branches: col = strip*512 + br*256 + idx2*128 + s*64
    eb = np.ones((128, 2048), dtype=BF16)
    for br in range(2):
        for h in range(H):
            st, i2 = _strip(h, br)
            e = np.exp(bias[h]).T.astype(BF16)  # [k, q]
            for s in range(2):
                c = st * 512 + br * 256 + i2 * 128 + s * 64
                eb[0:64, c:c + 64] = e
                eb[64:128, c:c + 64] = e

    pt = np.zeros((2, 192, 192), dtype=BF16)
    pt[0] = np.asarray(inputs["proj_sa_w"], dtype=np.float64).T.astype(BF16)
    pt[1] = np.asarray(inputs["proj_ca_w"], dtype=np.float64).T.astype(BF16)

    ident = np.eye(128, dtype=BF16)

    lam_sa = _sigmoid_clip(inputs["lambda_sa"])
    lam_ca = _sigmoid_clip(inputs["lambda_ca"])

    zs = []
    for c in range(N_CORES):
        z = np.concatenate([x[c * NWC:(c + 1) * NWC], y[c * NWC:(c + 1) * NWC]],
                           axis=-1)
        zT = z.reshape(NWC * N, 384).T.astype(BF16).reshape(3, 128, TOK_C)
        zs.append(np.ascontiguousarray(zT))

    shared = {"wfm": wfm, "wv": wv, "eb": eb, "pt": pt, "ident": ident}
    return shared, zs, (lam_sa, lam_ca)


def _build_nc(lam_sa, lam_ca, nwc=NWC):
    n_groups = nwc // GROUP
    tok_c = nwc * N
    nc = bacc.Bacc(None, target_bir_lowering=False)
    bf = mybir.dt.bfloat16
    f32 = mybir.dt.float32
    Exp = mybir.ActivationFunctionType.Exp

    zt_d = nc.declare_dram_parameter("zt", [3, 128, tok_c], bf, isOutput=False)
    wfm_d = nc.declare_dram_parameter("wfm", [2, 6, 3, 128, 128], bf, isOutput=False)
    wv_d = nc.declare_dram_parameter("wv", [3, 128, 768], bf, isOutput=False)
    eb_d = nc.declare_dram_parameter("eb", [128, 2048], bf, isOutput=False)
    pt_d = nc.declare_dram_parameter("pt", [2, 192, 192], bf, isOutput=False)
    id_d = nc.declare_dram_parameter("ident", [128, 128], bf, isOutput=False)
    out_d = nc.declare_dram_parameter("outT", [4, nwc, 192, N], f32, isOutput=True)

    lam = (lam_sa, lam_ca)

    with TileContext(nc) as tc:
        with (
            tc.tile_pool(name="const", bufs=1) as cpool,
            tc.tile_pool(name="zin", bufs=2) as zpool,
            tc.tile_pool(name="fm", bufs=2) as fmpool,
            tc.tile_pool(name="vt", bufs=2) as vpool,
            tc.tile_pool(name="escore", bufs=2) as epool,
            tc.tile_pool(name="small", bufs=2) as spool,
            tc.tile_pool(name="otile", bufs=2) as opool,
            tc.tile_pool(name="ps_lin", bufs=1, space="PSUM") as ps_lin,
            tc.tile_pool(name="ps_sc", bufs=1, space="PSUM") as ps_sc,
            tc.tile_pool(name="ps_tr", bufs=1, space="PSUM") as ps_tr,
            tc.tile_pool(name="ps_u", bufs=1, space="PSUM") as ps_u,
        ):
            # ---- constants ----
            wfm_t = [[[None] * 3 for _ in range(6)] for _ in range(2)]
            for s in range(2):
                for oc in range(6):
                    for kc in range(3):
                        t = cpool.tile([128, 128], bf, tag=f"wfm{s}{oc}{kc}")
                        nc.sync.dma_start(out=t[:], in_=wfm_d[s, oc, kc])
                        wfm_t[s][oc][kc] = t
            wv_t = []
            for kc in range(3):
                t = cpool.tile([128, 768], bf, tag=f"wv{kc}")
                nc.sync.dma_start(out=t[:], in_=wv_d[kc])
                wv_t.append(t)
            eb_t = cpool.tile([128, 2048], bf, tag="eb")
            nc.sync.dma_start(out=eb_t[:], in_=eb_d[:, :])
            pt_t = []
            for br in range(2):
                pk = []
                for kc in range(2):
                    t = cpool.tile([96, 192], bf, tag=f"pt{br}{kc}")
                    nc.sync.dma_start(out=t[:], in_=pt_d[br, kc * 96:(kc + 1) * 96, :])
                    pk.append(t)
                pt_t.append(pk)
            id_t = cpool.tile([128, 128], bf, tag="ident")
            nc.sync.dma_start(out=id_t[:], in_=id_d[:, :])

            for g in range(n_groups):
                tok0 = g * GROUP * N
                T = GROUP * N  # 512
                zt = []
                for kc in range(3):
                    t = zpool.tile([128, T], bf, tag=f"z{kc}")
                    nc.sync.dma_start(out=t[:], in_=zt_d[kc, :, tok0:tok0 + T])
                    zt.append(t)

                # ---- front-end q/k feature-major ----
                fm = [[None] * 6 for _ in range(2)]
                for s in range(2):
                    for oc in range(6):
                        ps = ps_lin.tile([128, T], f32, tag="lin")
                        for kc in range(3):
                            nc.tensor.matmul(ps[:], wfm_t[s][oc][kc][:], zt[kc][:],
                                             start=(kc == 0), stop=(kc == 2))
                        sb = fmpool.tile([128, T], bf, tag=f"fm{s}{oc}")
                        nc.any.tensor_copy(sb[:], ps[:])
                        fm[s][oc] = sb

                for tb in range(UNITS_PER_GROUP):
                    c0 = tb * 128
                    # ---- v token-major for this unit (2 windows) ----
                    vps = ps_lin.tile([128, 1024], f32, tag="lin")
                    for kc in range(3):
                        nc.tensor.matmul(vps[:, 0:384], zt[kc][:, c0:c0 + 128],
                                         wv_t[kc][:, 0:384],
                                         start=(kc == 0), stop=(kc == 2))
                        nc.tensor.matmul(vps[:, 512:896], zt[kc][:, c0:c0 + 128],
                                         wv_t[kc][:, 384:768],
                                         start=(kc == 0), stop=(kc == 2))
                    # v tiles [128, 6*33] per (br, s) with ones col
                    vt = [[None, None], [None, None]]
                    for br in range(2):
                        for s in range(2):
                            t = vpool.tile([128, 6 * 33], bf, tag=f"v{br}{s}")
                            tv = t[:].rearrange("p (h c) -> p h c", c=33)
                            src = vps[:, s * 512 + br * 192:s * 512 + br * 192 + 192]
                            nc.any.tensor_copy(
                                tv[:, :, 0:32],
                                src.rearrange("p (h c) -> p h c", c=32))
                            nc.vector.memset(tv[:, :, 32:33], 1.0)
                            vt[br][s] = t

                    # ---- scores, BOTH branches in one psum phase ----
                    # col = strip*512 + br*256 + idx2*128 + s*64 (w on partitions)
                    sc = ps_sc.tile([128, 2048], f32, tag="scpr")
                    filled = set()
                    for br in range(2):
                        for h in range(H):
                            if h < 4:
                                qoc, koc = (0, 1) if br == 0 else (4, 5)
                            else:
                                qoc, koc = 2, 3
                            st, i2 = _strip(h, br)
                            off = 32 * st
                            for s in range(2):
                                pc = st * 512 + br * 256 + i2 * 128 + s * 64
                                for w in range(2):
                                    cols = slice(c0 + w * 64, c0 + w * 64 + 64)
                                    q_ap = fm[s][qoc][off:off + 32, cols]
                                    k_ap = fm[s][koc][off:off + 32, cols]
                                    nc.tensor.matmul(
                                        sc[w * 64:w * 64 + 64, pc:pc + 64],
                                        k_ap, q_ap, start=True, stop=True,
                                        tile_position=(off, w * 64))
                                    filled.add(pc)
                    for st in range(4):
                        off = 32 * st
                        for slot in range(8):
                            pc = st * 512 + slot * 64
                            if pc in filled:
                                continue
                            for w in range(2):
                                cols = slice(c0 + w * 64, c0 + w * 64 + 64)
                                d_ap = fm[0][0][off:off + 32, cols]
                                nc.tensor.matmul(
                                    sc[w * 64:w * 64 + 64, pc:pc + 64],
                                    d_ap, d_ap, start=True, stop=True,
                                    tile_position=(off, w * 64))
                    # ---- exp + bias, one pass for both branches ----
                    ex = epool.tile([128, 2048], bf, tag="ex")
                    nc.scalar.activation(ex[:], sc[:], Exp)
                    ebx = epool.tile([128, 2048], bf, tag="ebx")
                    nc.vector.tensor_mul(ebx[:], ex[:], eb_t[:])

                    # ---- AV both branches: pr bank pair per (br, h-half) ----
                    pr = ps_sc.tile([128, 2048], f32, tag="scpr")
                    for br in range(2):
                        for h in range(H):
                            st, i2 = _strip(h, br)
                            pcq = st * 512 + br * 256 + i2 * 128
                            base = ((h % 3) * 132 + (512 if h >= 3 else 0)
                                    + 1024 * br)
                            for w in range(2):
                                et = ebx[w * 64:w * 64 + 64, pcq:pcq + 64]
                                er = ebx[w * 64:w * 64 + 64, pcq + 64:pcq + 128]
                                rows = slice(w * 64, w * 64 + 64)
                                vt_sl = vt[br][0][rows].rearrange(
                                    "p (h c) -> p h c", c=33)[:, h, :]
                                vr_sl = vt[br][1][rows].rearrange(
                                    "p (h c) -> p h c", c=33)[:, h, :]
                                o = w * 64
                                tp = (w * 64, w * 64)
                                for j, (ee, vv) in enumerate(
                                        [(et, vt_sl), (er, vt_sl),
                                         (er, vr_sl), (et, vr_sl)]):
                                    nc.tensor.matmul(
                                        pr[o:o + 64,
                                           base + 33 * j:base + 33 * j + 33],
                                        ee, vv, start=True, stop=True,
                                        tile_position=tp)
                    for br in range(2):
                        # ---- recips (R at col 32 of each 33-block) ----
                        pb = 1024 * br
                        prv0 = pr[:, pb:pb + 396].rearrange(
                            "p (b c) -> p b c", c=33)
                        prv1 = pr[:, pb + 512:pb + 908].rearrange(
                            "p (b c) -> p b c", c=33)
                        rec = spool.tile([128, 24], f32, tag="rec")
                        nc.vector.reciprocal(rec[:, 0:12], prv0[:, :, 32])
                        nc.vector.reciprocal(rec[:, 12:24], prv1[:, :, 32])
                        recl = spool.tile([128, 24], f32, tag="recl")
                        nc.vector.tensor_scalar_mul(recl[:], rec[:], float(lam[br]))

                        # ---- normalize + combine ----
                        # per half: blocks A,D,B,C per head (3 heads/half)
                        tA = opool.tile([128, 384], f32, tag="tA")
                        tD = opool.tile([128, 384], f32, tag="tD")
                        av = tA[:].rearrange("p (h c) -> p h c", c=32)
                        dv = tD[:].rearrange("p (h c) -> p h c", c=32)
                        for half, prv in enumerate([prv0, prv1]):
                            pa = prv[:, :, 0:32].rearrange(
                                "p (h f) c -> p h f c", f=4)
                            rc4 = rec[:, 12 * half:12 * half + 12].rearrange(
                                "p (h f) -> p h f", f=4)
                            rl4 = recl[:, 12 * half:12 * half + 12].rearrange(
                                "p (h f) -> p h f", f=4)
                            ha = 3 * half
                            nc.vector.tensor_mul(
                                av[:, ha:ha + 3, :], pa[:, :, 0, :],
                                rc4[:, :, 0:1].broadcast_to([128, 3, 32]))
                            nc.vector.tensor_mul(
                                dv[:, ha:ha + 3, :], pa[:, :, 1, :],
                                rl4[:, :, 1:2].broadcast_to([128, 3, 32]))
                            nc.vector.tensor_mul(
                                av[:, 6 + ha:6 + ha + 3, :], pa[:, :, 2, :],
                                rc4[:, :, 2:3].broadcast_to([128, 3, 32]))
                            nc.vector.tensor_mul(
                                dv[:, 6 + ha:6 + ha + 3, :], pa[:, :, 3, :],
                                rl4[:, :, 3:4].broadcast_to([128, 3, 32]))
                        oc_t = opool.tile([128, 384], bf, tag="oc")
                        nc.vector.tensor_sub(oc_t[:], tA[:], tD[:])

                        # ---- transpose to feature-major ----
                        trp = ps_tr.tile([128, 512], bf, tag="trp")
                        for ch in range(4):
                            nc.tensor.transpose(
                                trp[0:96, ch * 128:(ch + 1) * 128],
                                oc_t[:, ch * 96:(ch + 1) * 96], id_t[:])
                        otT = opool.tile([96, 512], bf, tag="otT")
                        nc.any.tensor_copy(otT[:], trp[0:96, :])

                        # ---- proj + output ----
                        for st in range(2):
                            ups = ps_u.tile([128, 256], f32, tag="u")
                            for ocn in range(2):
                                for kc in range(2):
                                    mv = otT[:, st * 256 + kc * 128:
                                             st * 256 + (kc + 1) * 128]
                                    wk = pt_t[br][kc]
                                    if ocn == 0:
                                        nc.tensor.matmul(
                                            ups[:, 0:128], wk[:, 0:128], mv,
                                            start=(kc == 0), stop=(kc == 1))
                                    else:
                                        nc.tensor.matmul(
                                            ups[0:64, 128:256], wk[:, 128:192], mv,
                                            start=(kc == 0), stop=(kc == 1))
                            ou = opool.tile([128, 256], f32, tag="ou")
                            nc.any.tensor_copy(ou[:, 0:128], ups[:, 0:128])
                            nc.any.tensor_copy(ou[0:64, 128:256],
                                               ups[0:64, 128:256])
                            qd = br * 2 + st
                            w1 = g * GROUP + tb * 2
                            for w in range(2):
                                nc.sync.dma_start(
                                    out=out_d[qd, w1 + w, 0:128, :],
                                    in_=ou[:, w * 64:w * 64 + 64])
                                nc.sync.dma_start(
                                    out=out_d[qd, w1 + w, 128:192, :],
                                    in_=ou[0:64, 128 + w * 64:128 + w * 64 + 64])
    nc.finalize()
    return nc


def _get_compiled(lam_sa, lam_ca):
    key = (round(lam_sa, 9), round(lam_ca, 9))
    if key not in _COMPILED:
        _COMPILED[key] = _build_nc(lam_sa, lam_ca)
    return _COMPILED[key]


def _run(nc, in_maps):
    from concourse.bass_utils import run_bass_kernel_spmd
    res = run_bass_kernel_spmd(nc, in_maps, list(range(N_CORES)))
    return res.results


def kernel(**inputs):
    shared, zs, (lam_sa, lam_ca) = _host_prep(inputs)
    nc = _get_compiled(lam_sa, lam_ca)
    in_maps = [{"zt": zs[c], **shared} for c in range(N_CORES)]
    results = _run(nc, in_maps)
    out = np.empty((4 * NW, N, DIM), dtype=F32)
    for c in range(N_CORES):
        o = results[c]["outT"]
        w0 = c * NWC
        # quarters: 0=sa_t, 1=sa_r; ca_out is interleaved (2b -> ca_t, 2b+1 -> ca_r)
        out[w0:w0 + NWC] = o[0].transpose(0, 2, 1)
        out[NW + w0:NW + w0 + NWC] = o[1].transpose(0, 2, 1)
        out[2 * NW + 2 * w0:2 * NW + 2 * (w0 + NWC):2] = o[2].transpose(0, 2, 1)
        out[2 * NW + 2 * w0 + 1:2 * NW + 2 * (w0 + NWC):2] = o[3].transpose(0, 2, 1)
    return out



# revision 14
# speedup vs baseline: 77.1476x; 77.1476x over previous
"""Trainium2 Bass kernel for DifferentialDualAttentionInteractiveBlock.

Self-contained: hardcodes shapes (nW=1024, N=64, DIM=192, H=6, d=32),
shards data-parallel over windows across 8 NeuronCores.
"""
import sys

sys.path.insert(0, "/opt/trn_rl_repo")

import numpy as np
import ml_dtypes

import concourse.bass as bass  # noqa: F401
import concourse.bacc as bacc
import concourse.mybir as mybir
from concourse.tile import TileContext

BF16 = ml_dtypes.bfloat16
F32 = np.float32

WS = 8
N = 64
DIM = 192
H = 6
HD = 32
SCALE = HD ** -0.5
N_CORES = 8
NW = 1024
NWC = NW // N_CORES
GROUP = 8
UNITS_PER_GROUP = GROUP // 2
N_GROUPS = NWC // GROUP
TOK_C = NWC * N

_COMPILED = {}


def _rel_pos_bias(rpb_table):
    coords = np.stack(np.meshgrid(np.arange(WS), np.arange(WS), indexing="ij"))
    cf = coords.reshape(2, -1)
    rc = (cf[:, :, None] - cf[:, None, :]).transpose(1, 2, 0).astype(np.int64)
    rc[:, :, 0] += WS - 1
    rc[:, :, 1] += WS - 1
    rc[:, :, 0] *= 2 * WS - 1
    idx = rc.sum(-1)
    bias = np.asarray(rpb_table)[idx.reshape(-1)].reshape(N, N, H)
    return bias.transpose(2, 0, 1).astype(np.float64)  # [H, q, k]


def _sigmoid_clip(lam):
    s = 1.0 / (1.0 + np.exp(-np.float64(lam)))
    return float(np.clip(s, 0.01, 0.99))


def _strip(h, br):
    """(row-strip, index-within-strip) for head h, branch br."""
    if h < 4:
        return h, 0
    return (h - 4) + 2 * br, 1


def _host_prep(inputs):
    x = np.asarray(inputs["x_windows"], dtype=F32)
    y = np.asarray(inputs["y_windows"], dtype=F32)

    qkv = np.asarray(inputs["sa_qkv_w"], dtype=np.float64)
    sa_ct = np.asarray(inputs["sa_ct_w"], dtype=np.float64)
    sa_cr = np.asarray(inputs["sa_cr_w"], dtype=np.float64)
    ca_q = np.asarray(inputs["ca_q_w"], dtype=np.float64)
    ca_kv = np.asarray(inputs["ca_kv_w"], dtype=np.float64)
    ca_ct = np.asarray(inputs["ca_ct_w"], dtype=np.float64)
    ca_cr = np.asarray(inputs["ca_cr_w"], dtype=np.float64)
    sa_enh = float(np.asarray(inputs["sa_enh"]))
    ca_enh = float(np.asarray(inputs["ca_enh"]))

    Wq_sa = qkv[0:DIM] * SCALE
    Wk_sa = qkv[DIM:2 * DIM]
    Wv_sa = qkv[2 * DIM:3 * DIM]
    Wq_ca = ca_q * SCALE
    Wk_ca = ca_kv[0:DIM]
    Wv_ca = ca_kv[DIM:2 * DIM]

    def ct(W, enh, cross):
        return np.concatenate([W, enh * (W @ cross)], axis=1)

    def cr(W, enh, cross):
        return np.concatenate([enh * (W @ cross), W], axis=1)

    W_fm_t = np.concatenate([
        ct(Wq_sa, sa_enh, sa_cr), ct(Wk_sa, sa_enh, sa_cr),
        ct(Wq_ca, ca_enh, ca_cr), ct(Wk_ca, ca_enh, ca_cr)], axis=0)
    W_fm_r = np.concatenate([
        cr(Wq_sa, sa_enh, sa_ct), cr(Wk_sa, sa_enh, sa_ct),
        cr(Wq_ca, ca_enh, ca_ct), cr(Wk_ca, ca_enh, ca_ct)], axis=0)
    # permute rows so q-head-h and k-head-h share partition offsets:
    # oc0=q_sa h0-3, oc1=k_sa h0-3, oc2=[q4s,q5s,q4c,q5c], oc3=[k4s,k5s,k4c,k5c],
    # oc4=q_ca h0-3, oc5=k_ca h0-3
    perm = np.concatenate([
        np.arange(0, 128), np.arange(192, 320),
        np.arange(128, 192), np.arange(384 + 128, 384 + 192),
        np.arange(320, 384), np.arange(576 + 128, 576 + 192),
        np.arange(384, 512), np.arange(576, 704)])
    W_fm_t = W_fm_t[perm]
    W_fm_r = W_fm_r[perm]
    wfm = np.zeros((2, 6, 3, 128, 128), dtype=BF16)
    for s, W in enumerate([W_fm_t, W_fm_r]):
        for oc in range(6):
            for kc in range(3):
                blk = W[oc * 128:(oc + 1) * 128, kc * 128:(kc + 1) * 128]
                wfm[s, oc, kc] = blk.T.astype(BF16)

    W_v_t = np.concatenate([ct(Wv_sa, sa_enh, sa_cr),
                            ct(Wv_ca, ca_enh, ca_cr)], axis=0)
    W_v_r = np.concatenate([cr(Wv_sa, sa_enh, sa_ct),
                            cr(Wv_ca, ca_enh, ca_ct)], axis=0)
    # wv[kc] : [128, 768] = [t-side (384) | r-side (384)]
    wv = np.zeros((3, 128, 768), dtype=BF16)
    for kc in range(3):
        wv[kc, :, 0:384] = W_v_t[:, kc * 128:(kc + 1) * 128].T.astype(BF16)
        wv[kc, :, 384:768] = W_v_r[:, kc * 128:(kc + 1) * 128].T.astype(BF16)

    bias = _rel_pos_bias(inputs["rpb_table"])
    # exp(bias), both branches: col = strip*512 + br*256 + idx2*128 + s*64
    eb = np.ones((128, 2048), dtype=BF16)
    for br in range(2):
        for h in range(H):
            st, i2 = _strip(h, br)
            e = np.exp(bias[h]).T.astype(BF16)  # [k, q]
            for s in range(2):
                c = st * 512 + br * 256 + i2 * 128 + s * 64
                eb[0:64, c:c + 64] = e
                eb[64:128, c:c + 64] = e

    pt = np.zeros((2, 192, 192), dtype=BF16)
    pt[0] = np.asarray(inputs["proj_sa_w"], dtype=np.float64).T.astype(BF16)
    pt[1] = np.asarray(inputs["proj_ca_w"], dtype=np.float64).T.astype(BF16)

    ident = np.eye(128, dtype=BF16)

    lam_sa = _sigmoid_clip(inputs["lambda_sa"])
    lam_ca = _sigmoid_clip(inputs["lambda_ca"])

    zs = []
    for c in range(N_CORES):
        z = np.concatenate([x[c * NWC:(c + 1) * NWC], y[c * NWC:(c + 1) * NWC]],
                           axis=-1)
        zT = z.reshape(NWC * N, 384).T.astype(BF16).reshape(3, 128, TOK_C)
        zs.append(np.ascontiguousarray(zT))

    shared = {"wfm": wfm, "wv": wv, "eb": eb, "pt": pt, "ident": ident}
    return shared, zs, (lam_sa, lam_ca)


def _build_nc(lam_sa, lam_ca, nwc=NWC):
    n_groups = nwc // GROUP
    tok_c = nwc * N
    nc = bacc.Bacc(None, target_bir_lowering=False)
    bf = mybir.dt.bfloat16
    f32 = mybir.dt.float32
    Exp = mybir.ActivationFunctionType.Exp

    zt_d = nc.declare_dram_parameter("zt", [3, 128, tok_c], bf, isOutput=False)
    wfm_d = nc.declare_dram_parameter("wfm", [2, 6, 3, 128, 128], bf, isOutput=False)
    wv_d = nc.declare_dram_parameter("wv", [3, 128, 768], bf, isOutput=False)
    eb_d = nc.declare_dram_parameter("eb", [128, 2048], bf, isOutput=False)
    pt_d = nc.declare_dram_parameter("pt", [2, 192, 192], bf, isOutput=False)
    id_d = nc.declare_dram_parameter("ident", [128, 128], bf, isOutput=False)
    out_d = nc.declare_dram_parameter("outT", [4, nwc, 192, N], f32, isOutput=True)

    lam = (lam_sa, lam_ca)

    with TileContext(nc) as tc:
        with (
            tc.tile_pool(name="const", bufs=1) as cpool,
            tc.tile_pool(name="zin", bufs=2) as zpool,
            tc.tile_pool(name="fm", bufs=2) as fmpool,
            tc.tile_pool(name="vt", bufs=2) as vpool,
            tc.tile_pool(name="escore", bufs=2) as epool,
            tc.tile_pool(name="small", bufs=2) as spool,
            tc.tile_pool(name="otile", bufs=2) as opool,
            tc.tile_pool(name="ps_lin", bufs=1, space="PSUM") as ps_lin,
            tc.tile_pool(name="ps_sc", bufs=1, space="PSUM") as ps_sc,
            tc.tile_pool(name="ps_tr", bufs=1, space="PSUM") as ps_tr,
            tc.tile_pool(name="ps_u", bufs=1, space="PSUM") as ps_u,
        ):
            # ---- constants ----
            wfm_t = [[[None] * 3 for _ in range(6)] for _ in range(2)]
            for s in range(2):
                for oc in range(6):
                    for kc in range(3):
                        t = cpool.tile([128, 128], bf, tag=f"wfm{s}{oc}{kc}")
                        nc.sync.dma_start(out=t[:], in_=wfm_d[s, oc, kc])
                        wfm_t[s][oc][kc] = t
            wv_t = []
            for kc in range(3):
                t = cpool.tile([128, 768], bf, tag=f"wv{kc}")
                nc.sync.dma_start(out=t[:], in_=wv_d[kc])
                wv_t.append(t)
            eb_t = cpool.tile([128, 2048], bf, tag="eb")
            nc.sync.dma_start(out=eb_t[:], in_=eb_d[:, :])
            pt_t = []
            for br in range(2):
                pk = []
                for kc in range(2):
                    t = cpool.tile([96, 192], bf, tag=f"pt{br}{kc}")
                    nc.sync.dma_start(out=t[:], in_=pt_d[br, kc * 96:(kc + 1) * 96, :])
                    pk.append(t)
                pt_t.append(pk)
            id_t = cpool.tile([128, 128], bf, tag="ident")
            nc.sync.dma_start(out=id_t[:], in_=id_d[:, :])

            for g in range(n_groups):
                tok0 = g * GROUP * N
                T = GROUP * N  # 512
                zt = []
                for kc in range(3):
                    t = zpool.tile([128, T], bf, tag=f"z{kc}")
                    nc.sync.dma_start(out=t[:], in_=zt_d[kc, :, tok0:tok0 + T])
                    zt.append(t)

                # ---- front-end q/k feature-major ----
                fm = [[None] * 6 for _ in range(2)]
                for s in range(2):
                    for oc in range(6):
                        ps = ps_lin.tile([128, T], f32, tag="lin")
                        for kc in range(3):
                            nc.tensor.matmul(ps[:], wfm_t[s][oc][kc][:], zt[kc][:],
                                             start=(kc == 0), stop=(kc == 2))
                        sb = fmpool.tile([128, T], bf, tag=f"fm{s}{oc}")
                        nc.any.tensor_copy(sb[:], ps[:])
                        fm[s][oc] = sb

                for tb in range(UNITS_PER_GROUP):
                    c0 = tb * 128
                    # ---- v token-major for this unit (2 windows) ----
                    vps = ps_lin.tile([128, 1024], f32, tag="lin")
                    for kc in range(3):
                        nc.tensor.matmul(vps[:, 0:384], zt[kc][:, c0:c0 + 128],
                                         wv_t[kc][:, 0:384],
                                         start=(kc == 0), stop=(kc == 2))
                        nc.tensor.matmul(vps[:, 512:896], zt[kc][:, c0:c0 + 128],
                                         wv_t[kc][:, 384:768],
                                         start=(kc == 0), stop=(kc == 2))
                    # v tiles [128, 6*33] per (br, s) with ones col
                    vt = [[None, None], [None, None]]
                    for br in range(2):
                        for s in range(2):
                            t = vpool.tile([128, 6 * 33], bf, tag=f"v{br}{s}")
                            tv = t[:].rearrange("p (h c) -> p h c", c=33)
                            src = vps[:, s * 512 + br * 192:s * 512 + br * 192 + 192]
                            nc.any.tensor_copy(
                                tv[:, :, 0:32],
                                src.rearrange("p (h c) -> p h c", c=32))
                            nc.vector.memset(tv[:, :, 32:33], 1.0)
                            vt[br][s] = t

                    # ---- scores, BOTH branches in one psum phase ----
                    # col = strip*512 + br*256 + idx2*128 + s*64 (w on partitions)
                    sc = ps_sc.tile([128, 2048], f32, tag="scpr")
                    filled = set()
                    for br in range(2):
                        for h in range(H):
                            if h < 4:
                                qoc, koc = (0, 1) if br == 0 else (4, 5)
                            else:
                                qoc, koc = 2, 3
                            st, i2 = _strip(h, br)
                            off = 32 * st
                            for s in range(2):
                                pc = st * 512 + br * 256 + i2 * 128 + s * 64
                                for w in range(2):
                                    cols = slice(c0 + w * 64, c0 + w * 64 + 64)
                                    q_ap = fm[s][qoc][off:off + 32, cols]
                                    k_ap = fm[s][koc][off:off + 32, cols]
                                    nc.tensor.matmul(
                                        sc[w * 64:w * 64 + 64, pc:pc + 64],
                                        k_ap, q_ap, start=True, stop=True,
                                        tile_position=(off, w * 64))
                                    filled.add(pc)
                    for st in range(4):
                        off = 32 * st
                        for slot in range(8):
                            pc = st * 512 + slot * 64
                            if pc in filled:
                                continue
                            for w in range(2):
                                cols = slice(c0 + w * 64, c0 + w * 64 + 64)
                                d_ap = fm[0][0][off:off + 32, cols]
                                nc.tensor.matmul(
                                    sc[w * 64:w * 64 + 64, pc:pc + 64],
                                    d_ap, d_ap, start=True, stop=True,
                                    tile_position=(off, w * 64))
                    # ---- exp + bias, one pass for both branches ----
                    ex = epool.tile([128, 2048], bf, tag="ex")
                    nc.scalar.activation(ex[:], sc[:], Exp)
                    ebx = epool.tile([128, 2048], bf, tag="ebx")
                    nc.vector.tensor_mul(ebx[:], ex[:], eb_t[:])

                    # ---- AV both branches: pr bank pair per (br, h-half) ----
                    pr = ps_sc.tile([128, 2048], f32, tag="scpr")
                    for br in range(2):
                        for h in range(H):
                            st, i2 = _strip(h, br)
                            pcq = st * 512 + br * 256 + i2 * 128
                            base = ((h % 3) * 132 + (512 if h >= 3 else 0)
                                    + 1024 * br)
                            for w in range(2):
                                et = ebx[w * 64:w * 64 + 64, pcq:pcq + 64]
                                er = ebx[w * 64:w * 64 + 64, pcq + 64:pcq + 128]
                                rows = slice(w * 64, w * 64 + 64)
                                vt_sl = vt[br][0][rows].rearrange(
                                    "p (h c) -> p h c", c=33)[:, h, :]
                                vr_sl = vt[br][1][rows].rearrange(
                                    "p (h c) -> p h c", c=33)[:, h, :]
                                o = w * 64
                                tp = (w * 64, w * 64)
                                for j, (ee, vv) in enumerate(
                                        [(et, vt_sl), (er, vt_sl),
                                         (er, vr_sl), (et, vr_sl)]):
                                    nc.tensor.matmul(
                                        pr[o:o + 64,
                                           base + 33 * j:base + 33 * j + 33],
                                        ee, vv, start=True, stop=True,
                                        tile_position=tp)
                    for br in range(2):
                        # ---- recips (R at col 32 of each 33-block) ----
                        pb = 1024 * br
                        prv0 = pr[:, pb:pb + 396].rearrange(
                            "p (b c) -> p b c", c=33)
                        prv1 = pr[:, pb + 512:pb + 908].rearrange(
                            "p (b c) -> p b c", c=33)
                        rec = spool.tile([128, 24], f32, tag="rec")
                        nc.vector.reciprocal(rec[:, 0:12], prv0[:, :, 32])
                        nc.vector.reciprocal(rec[:, 12:24], prv1[:, :, 32])
                        recl = spool.tile([128, 24], f32, tag="recl")
                        nc.vector.tensor_scalar_mul(recl[:], rec[:], float(lam[br]))

                        # ---- normalize + combine ----
                        # per half: blocks A,D,B,C per head (3 heads/half)
                        tA = opool.tile([128, 384], f32, tag="tA")
                        tD = opool.tile([128, 384], f32, tag="tD")
                        av = tA[:].rearrange("p (h c) -> p h c", c=32)
                        dv = tD[:].rearrange("p (h c) -> p h c", c=32)
                        for half, prv in enumerate([prv0, prv1]):
                            pa = prv[:, :, 0:32].rearrange(
                                "p (h f) c -> p h f c", f=4)
                            rc4 = rec[:, 12 * half:12 * half + 12].rearrange(
                                "p (h f) -> p h f", f=4)
                            rl4 = recl[:, 12 * half:12 * half + 12].rearrange(
                                "p (h f) -> p h f", f=4)
                            ha = 3 * half
                            nc.vector.tensor_mul(
                                av[:, ha:ha + 3, :], pa[:, :, 0, :],
                                rc4[:, :, 0:1].broadcast_to([128, 3, 32]))
                            nc.vector.tensor_mul(
                                dv[:, ha:ha + 3, :], pa[:, :, 1, :],
                                rl4[:, :, 1:2].broadcast_to([128, 3, 32]))
                            nc.vector.tensor_mul(
                                av[:, 6 + ha:6 + ha + 3, :], pa[:, :, 2, :],
                                rc4[:, :, 2:3].broadcast_to([128, 3, 32]))
                            nc.vector.tensor_mul(
                                dv[:, 6 + ha:6 + ha + 3, :], pa[:, :, 3, :],
                                rl4[:, :, 3:4].broadcast_to([128, 3, 32]))
                        oc_t = opool.tile([128, 384], bf, tag="oc")
                        nc.vector.tensor_sub(oc_t[:], tA[:], tD[:])

                        # ---- transpose to feature-major ----
                        trp = ps_tr.tile([128, 512], bf, tag="trp")
                        for ch in range(4):
                            nc.tensor.transpose(
                                trp[0:96, ch * 128:(ch + 1) * 128],
                                oc_t[:, ch * 96:(ch + 1) * 96], id_t[:])
                        otT = opool.tile([96, 512], bf, tag="otT")
                        nc.any.tensor_copy(otT[:], trp[0:96, :])

                        # ---- proj + output ----
                        for st in range(2):
                            ups = ps_u.tile([128, 256], f32, tag="u")
                            for ocn in range(2):
                                for kc in range(2):
                                    mv = otT[:, st * 256 + kc * 128:
                                             st * 256 + (kc + 1) * 128]
                                    wk = pt_t[br][kc]
                                    if ocn == 0:
                                        nc.tensor.matmul(
                                            ups[:, 0:128], wk[:, 0:128], mv,
                                            start=(kc == 0), stop=(kc == 1))
                                    else:
                                        nc.tensor.matmul(
                                            ups[0:64, 128:256], wk[:, 128:192], mv,
                                            start=(kc == 0), stop=(kc == 1))
                            ou = opool.tile([128, 256], f32, tag="ou")
                            nc.any.tensor_copy(ou[:, 0:128], ups[:, 0:128])
                            nc.any.tensor_copy(ou[0:64, 128:256],
                                               ups[0:64, 128:256])
                            qd = br * 2 + st
                            w1 = g * GROUP + tb * 2
                            for w in range(2):
                                nc.sync.dma_start(
                                    out=out_d[qd, w1 + w, 0:128, :],
                                    in_=ou[:, w * 64:w * 64 + 64])
                                nc.sync.dma_start(
                                    out=out_d[qd, w1 + w, 128:192, :],
                                    in_=ou[0:64, 128 + w * 64:128 + w * 64 + 64])
    nc.finalize()
    return nc


def _get_compiled(lam_sa, lam_ca):
    key = (round(lam_sa, 9), round(lam_ca, 9))
    if key not in _COMPILED:
        _COMPILED[key] = _build_nc(lam_sa, lam_ca)
    return _COMPILED[key]


def _run(nc, in_maps):
    from concourse.bass_utils import run_bass_kernel_spmd
    res = run_bass_kernel_spmd(nc, in_maps, list(range(N_CORES)))
    return res.results


def kernel(**inputs):
    shared, zs, (lam_sa, lam_ca) = _host_prep(inputs)
    nc = _get_compiled(lam_sa, lam_ca)
    in_maps = [{"zt": zs[c], **shared} for c in range(N_CORES)]
    results = _run(nc, in_maps)
    out = np.empty((4 * NW, N, DIM), dtype=F32)
    for c in range(N_CORES):
        o = results[c]["outT"]
        w0 = c * NWC
        # quarters: 0=sa_t, 1=sa_r; ca_out is interleaved (2b -> ca_t, 2b+1 -> ca_r)
        out[w0:w0 + NWC] = o[0].transpose(0, 2, 1)
        out[NW + w0:NW + w0 + NWC] = o[1].transpose(0, 2, 1)
        out[2 * NW + 2 * w0:2 * NW + 2 * (w0 + NWC):2] = o[2].transpose(0, 2, 1)
        out[2 * NW + 2 * w0 + 1:2 * NW + 2 * (w0 + NWC):2] = o[3].transpose(0, 2, 1)
    return out



# revision 16
# speedup vs baseline: 86.1343x; 1.1165x over previous
"""Trainium2 Bass kernel for DifferentialDualAttentionInteractiveBlock.

Self-contained: hardcodes shapes (nW=1024, N=64, DIM=192, H=6, d=32),
shards data-parallel over windows across 8 NeuronCores.
"""
import sys

sys.path.insert(0, "/opt/trn_rl_repo")

import numpy as np
import ml_dtypes

import concourse.bass as bass  # noqa: F401
import concourse.bacc as bacc
import concourse.mybir as mybir
from concourse.tile import TileContext

BF16 = ml_dtypes.bfloat16
F32 = np.float32

WS = 8
N = 64
DIM = 192
H = 6
HD = 32
SCALE = HD ** -0.5
N_CORES = 8
NW = 1024
NWC = NW // N_CORES
GROUP = 8
UNITS_PER_GROUP = GROUP // 2
N_GROUPS = NWC // GROUP
TOK_C = NWC * N

_COMPILED = {}


def _rel_pos_bias(rpb_table):
    coords = np.stack(np.meshgrid(np.arange(WS), np.arange(WS), indexing="ij"))
    cf = coords.reshape(2, -1)
    rc = (cf[:, :, None] - cf[:, None, :]).transpose(1, 2, 0).astype(np.int64)
    rc[:, :, 0] += WS - 1
    rc[:, :, 1] += WS - 1
    rc[:, :, 0] *= 2 * WS - 1
    idx = rc.sum(-1)
    bias = np.asarray(rpb_table)[idx.reshape(-1)].reshape(N, N, H)
    return bias.transpose(2, 0, 1).astype(np.float64)  # [H, q, k]


def _sigmoid_clip(lam):
    s = 1.0 / (1.0 + np.exp(-np.float64(lam)))
    return float(np.clip(s, 0.01, 0.99))


def _strip(h, br):
    """(row-strip, index-within-strip) for head h, branch br."""
    if h < 4:
        return h, 0
    return (h - 4) + 2 * br, 1


def _host_prep(inputs):
    x = np.asarray(inputs["x_windows"], dtype=F32)
    y = np.asarray(inputs["y_windows"], dtype=F32)

    qkv = np.asarray(inputs["sa_qkv_w"], dtype=np.float64)
    sa_ct = np.asarray(inputs["sa_ct_w"], dtype=np.float64)
    sa_cr = np.asarray(inputs["sa_cr_w"], dtype=np.float64)
    ca_q = np.asarray(inputs["ca_q_w"], dtype=np.float64)
    ca_kv = np.asarray(inputs["ca_kv_w"], dtype=np.float64)
    ca_ct = np.asarray(inputs["ca_ct_w"], dtype=np.float64)
    ca_cr = np.asarray(inputs["ca_cr_w"], dtype=np.float64)
    sa_enh = float(np.asarray(inputs["sa_enh"]))
    ca_enh = float(np.asarray(inputs["ca_enh"]))

    Wq_sa = qkv[0:DIM] * SCALE
    Wk_sa = qkv[DIM:2 * DIM]
    Wv_sa = qkv[2 * DIM:3 * DIM]
    Wq_ca = ca_q * SCALE
    Wk_ca = ca_kv[0:DIM]
    Wv_ca = ca_kv[DIM:2 * DIM]

    def ct(W, enh, cross):
        return np.concatenate([W, enh * (W @ cross)], axis=1)

    def cr(W, enh, cross):
        return np.concatenate([enh * (W @ cross), W], axis=1)

    W_fm_t = np.concatenate([
        ct(Wq_sa, sa_enh, sa_cr), ct(Wk_sa, sa_enh, sa_cr),
        ct(Wq_ca, ca_enh, ca_cr), ct(Wk_ca, ca_enh, ca_cr)], axis=0)
    W_fm_r = np.concatenate([
        cr(Wq_sa, sa_enh, sa_ct), cr(Wk_sa, sa_enh, sa_ct),
        cr(Wq_ca, ca_enh, ca_ct), cr(Wk_ca, ca_enh, ca_ct)], axis=0)
    # permute rows so q-head-h and k-head-h share partition offsets:
    # oc0=q_sa h0-3, oc1=k_sa h0-3, oc2=[q4s,q5s,q4c,q5c], oc3=[k4s,k5s,k4c,k5c],
    # oc4=q_ca h0-3, oc5=k_ca h0-3
    perm = np.concatenate([
        np.arange(0, 128), np.arange(192, 320),
        np.arange(128, 192), np.arange(384 + 128, 384 + 192),
        np.arange(320, 384), np.arange(576 + 128, 576 + 192),
        np.arange(384, 512), np.arange(576, 704)])
    W_fm_t = W_fm_t[perm]
    W_fm_r = W_fm_r[perm]
    wfm = np.zeros((2, 6, 3, 128, 128), dtype=BF16)
    for s, W in enumerate([W_fm_t, W_fm_r]):
        for oc in range(6):
            for kc in range(3):
                blk = W[oc * 128:(oc + 1) * 128, kc * 128:(kc + 1) * 128]
                wfm[s, oc, kc] = blk.T.astype(BF16)

    W_v_t = np.concatenate([ct(Wv_sa, sa_enh, sa_cr),
                            ct(Wv_ca, ca_enh, ca_cr)], axis=0)
    W_v_r = np.concatenate([cr(Wv_sa, sa_enh, sa_ct),
                            cr(Wv_ca, ca_enh, ca_ct)], axis=0)
    # wv[kc] : [128, 768] = [t-side (384) | r-side (384)]
    wv = np.zeros((3, 128, 768), dtype=BF16)
    for kc in range(3):
        wv[kc, :, 0:384] = W_v_t[:, kc * 128:(kc + 1) * 128].T.astype(BF16)
        wv[kc, :, 384:768] = W_v_r[:, kc * 128:(kc + 1) * 128].T.astype(BF16)

    bias = _rel_pos_bias(inputs["rpb_table"])
    # exp(bias), both branches: col = strip*512 + br*256 + idx2*128 + s*64
    eb = np.ones((128, 2048), dtype=BF16)
    for br in range(2):
        for h in range(H):
            st, i2 = _strip(h, br)
            e = np.exp(bias[h]).T.astype(BF16)  # [k, q]
            for s in range(2):
                c = st * 512 + br * 256 + i2 * 128 + s * 64
                eb[0:64, c:c + 64] = e
                eb[64:128, c:c + 64] = e

    pt = np.zeros((2, 192, 192), dtype=BF16)
    pt[0] = np.asarray(inputs["proj_sa_w"], dtype=np.float64).T.astype(BF16)
    pt[1] = np.asarray(inputs["proj_ca_w"], dtype=np.float64).T.astype(BF16)

    ident = np.eye(128, dtype=BF16)

    lam_sa = _sigmoid_clip(inputs["lambda_sa"])
    lam_ca = _sigmoid_clip(inputs["lambda_ca"])

    zs = []
    for c in range(N_CORES):
        z = np.concatenate([x[c * NWC:(c + 1) * NWC], y[c * NWC:(c + 1) * NWC]],
                           axis=-1)
        zT = z.reshape(NWC * N, 384).T.astype(BF16).reshape(3, 128, TOK_C)
        zs.append(np.ascontiguousarray(zT))

    shared = {"wfm": wfm, "wv": wv, "eb": eb, "pt": pt, "ident": ident}
    return shared, zs, (lam_sa, lam_ca)


def _build_nc(lam_sa, lam_ca, nwc=NWC):
    n_groups = nwc // GROUP
    tok_c = nwc * N
    nc = bacc.Bacc(None, target_bir_lowering=False)
    bf = mybir.dt.bfloat16
    f32 = mybir.dt.float32
    Exp = mybir.ActivationFunctionType.Exp

    zt_d = nc.declare_dram_parameter("zt", [3, 128, tok_c], bf, isOutput=False)
    wfm_d = nc.declare_dram_parameter("wfm", [2, 6, 3, 128, 128], bf, isOutput=False)
    wv_d = nc.declare_dram_parameter("wv", [3, 128, 768], bf, isOutput=False)
    eb_d = nc.declare_dram_parameter("eb", [128, 2048], bf, isOutput=False)
    pt_d = nc.declare_dram_parameter("pt", [2, 192, 192], bf, isOutput=False)
    id_d = nc.declare_dram_parameter("ident", [128, 128], bf, isOutput=False)
    out_d = nc.declare_dram_parameter("outT", [4, nwc, 192, N], f32, isOutput=True)

    lam = (lam_sa, lam_ca)

    with TileContext(nc) as tc:
        with (
            tc.tile_pool(name="const", bufs=1) as cpool,
            tc.tile_pool(name="zin", bufs=2) as zpool,
            tc.tile_pool(name="fm", bufs=2) as fmpool,
            tc.tile_pool(name="vt", bufs=2) as vpool,
            tc.tile_pool(name="escore", bufs=2) as epool,
            tc.tile_pool(name="small", bufs=2) as spool,
            tc.tile_pool(name="otile", bufs=2) as opool,
            tc.tile_pool(name="ps_lin", bufs=1, space="PSUM") as ps_lin,
            tc.tile_pool(name="ps_sc", bufs=1, space="PSUM") as ps_sc,
            tc.tile_pool(name="ps_tr", bufs=1, space="PSUM") as ps_tr,
            tc.tile_pool(name="ps_u", bufs=1, space="PSUM") as ps_u,
        ):
            # ---- constants ----
            wfm_t = [[[None] * 3 for _ in range(6)] for _ in range(2)]
            for s in range(2):
                for oc in range(6):
                    for kc in range(3):
                        t = cpool.tile([128, 128], bf, tag=f"wfm{s}{oc}{kc}")
                        nc.sync.dma_start(out=t[:], in_=wfm_d[s, oc, kc])
                        wfm_t[s][oc][kc] = t
            wv_t = []
            for kc in range(3):
                t = cpool.tile([128, 768], bf, tag=f"wv{kc}")
                nc.sync.dma_start(out=t[:], in_=wv_d[kc])
                wv_t.append(t)
            eb_t = cpool.tile([128, 2048], bf, tag="eb")
            nc.sync.dma_start(out=eb_t[:], in_=eb_d[:, :])
            pt_t = []
            for br in range(2):
                pk = []
                for kc in range(2):
                    t = cpool.tile([96, 192], bf, tag=f"pt{br}{kc}")
                    nc.sync.dma_start(out=t[:], in_=pt_d[br, kc * 96:(kc + 1) * 96, :])
                    pk.append(t)
                pt_t.append(pk)
            id_t = cpool.tile([128, 128], bf, tag="ident")
            nc.sync.dma_start(out=id_t[:], in_=id_d[:, :])

            for g in range(n_groups):
                tok0 = g * GROUP * N
                T = GROUP * N  # 512
                zt = []
                for kc in range(3):
                    t = zpool.tile([128, T], bf, tag=f"z{kc}")
                    nc.sync.dma_start(out=t[:], in_=zt_d[kc, :, tok0:tok0 + T])
                    zt.append(t)

                # ---- front-end q/k feature-major ----
                fm = [[None] * 6 for _ in range(2)]
                for s in range(2):
                    for oc in range(6):
                        ps = ps_lin.tile([128, T], f32, tag="lin")
                        for kc in range(3):
                            nc.tensor.matmul(ps[:], wfm_t[s][oc][kc][:], zt[kc][:],
                                             start=(kc == 0), stop=(kc == 2))
                        sb = fmpool.tile([128, T], bf, tag=f"fm{s}{oc}")
                        nc.any.tensor_copy(sb[:], ps[:])
                        fm[s][oc] = sb

                for tb in range(UNITS_PER_GROUP):
                    c0 = tb * 128
                    # ---- v token-major for this unit (2 windows) ----
                    vps = ps_lin.tile([128, 1024], f32, tag="lin")
                    for kc in range(3):
                        nc.tensor.matmul(vps[:, 0:384], zt[kc][:, c0:c0 + 128],
                                         wv_t[kc][:, 0:384],
                                         start=(kc == 0), stop=(kc == 2))
                        nc.tensor.matmul(vps[:, 512:896], zt[kc][:, c0:c0 + 128],
                                         wv_t[kc][:, 384:768],
                                         start=(kc == 0), stop=(kc == 2))
                    # v tiles [128, 6*33] per (br, s) with ones col
                    vt = [[None, None], [None, None]]
                    for br in range(2):
                        for s in range(2):
                            t = vpool.tile([128, 6 * 33], bf, tag=f"v{br}{s}")
                            tv = t[:].rearrange("p (h c) -> p h c", c=33)
                            src = vps[:, s * 512 + br * 192:s * 512 + br * 192 + 192]
                            nc.any.tensor_copy(
                                tv[:, :, 0:32],
                                src.rearrange("p (h c) -> p h c", c=32))
                            nc.vector.memset(tv[:, :, 32:33], 1.0)
                            vt[br][s] = t

                    # ---- scores, BOTH branches in one psum phase ----
                    # col = strip*512 + br*256 + idx2*128 + s*64 (w on partitions)
                    sc = ps_sc.tile([128, 2048], f32, tag="scpr")
                    filled = set()
                    for br in range(2):
                        for h in range(H):
                            if h < 4:
                                qoc, koc = (0, 1) if br == 0 else (4, 5)
                            else:
                                qoc, koc = 2, 3
                            st, i2 = _strip(h, br)
                            off = 32 * st
                            for s in range(2):
                                pc = st * 512 + br * 256 + i2 * 128 + s * 64
                                for w in range(2):
                                    cols = slice(c0 + w * 64, c0 + w * 64 + 64)
                                    q_ap = fm[s][qoc][off:off + 32, cols]
                                    k_ap = fm[s][koc][off:off + 32, cols]
                                    nc.tensor.matmul(
                                        sc[w * 64:w * 64 + 64, pc:pc + 64],
                                        k_ap, q_ap, start=True, stop=True,
                                        tile_position=(off, w * 64))
                                    filled.add(pc)
                    for st in range(4):
                        off = 32 * st
                        for slot in range(8):
                            pc = st * 512 + slot * 64
                            if pc in filled:
                                continue
                            for w in range(2):
                                cols = slice(c0 + w * 64, c0 + w * 64 + 64)
                                d_ap = fm[0][0][off:off + 32, cols]
                                nc.tensor.matmul(
                                    sc[w * 64:w * 64 + 64, pc:pc + 64],
                                    d_ap, d_ap, start=True, stop=True,
                                    tile_position=(off, w * 64))
                    # ---- exp + bias, one pass for both branches ----
                    ex = epool.tile([128, 2048], bf, tag="ex")
                    nc.scalar.activation(ex[:], sc[:], Exp)
                    ebx = epool.tile([128, 2048], bf, tag="ebx")
                    nc.vector.tensor_mul(ebx[:], ex[:], eb_t[:])

                    # ---- AV both branches: pr bank pair per (br, h-half) ----
                    pr = ps_sc.tile([128, 2048], f32, tag="scpr")
                    for br in range(2):
                        for h in range(H):
                            st, i2 = _strip(h, br)
                            pcq = st * 512 + br * 256 + i2 * 128
                            base = ((h % 3) * 132 + (512 if h >= 3 else 0)
                                    + 1024 * br)
                            for w in range(2):
                                et = ebx[w * 64:w * 64 + 64, pcq:pcq + 64]
                                er = ebx[w * 64:w * 64 + 64, pcq + 64:pcq + 128]
                                rows = slice(w * 64, w * 64 + 64)
                                vt_sl = vt[br][0][rows].rearrange(
                                    "p (h c) -> p h c", c=33)[:, h, :]
                                vr_sl = vt[br][1][rows].rearrange(
                                    "p (h c) -> p h c", c=33)[:, h, :]
                                o = w * 64
                                tp = (w * 64, w * 64)
                                for j, (ee, vv) in enumerate(
                                        [(et, vt_sl), (er, vt_sl),
                                         (er, vr_sl), (et, vr_sl)]):
                                    nc.tensor.matmul(
                                        pr[o:o + 64,
                                           base + 33 * j:base + 33 * j + 33],
                                        ee, vv, start=True, stop=True,
                                        tile_position=tp)
                    for br in range(2):
                        # ---- recips (R at col 32 of each 33-block) ----
                        pb = 1024 * br
                        prv0 = pr[:, pb:pb + 396].rearrange(
                            "p (b c) -> p b c", c=33)
                        prv1 = pr[:, pb + 512:pb + 908].rearrange(
                            "p (b c) -> p b c", c=33)
                        rec = spool.tile([128, 24], f32, tag="rec")
                        nc.vector.reciprocal(rec[:, 0:12], prv0[:, :, 32])
                        nc.vector.reciprocal(rec[:, 12:24], prv1[:, :, 32])
                        recl = spool.tile([128, 24], f32, tag="recl")
                        nc.vector.tensor_scalar_mul(recl[:], rec[:], float(lam[br]))

                        # ---- normalize + combine ----
                        # per half: blocks A,D,B,C per head (3 heads/half)
                        tA = opool.tile([128, 384], f32, tag="tA")
                        tD = opool.tile([128, 384], f32, tag="tD")
                        av = tA[:].rearrange("p (h c) -> p h c", c=32)
                        dv = tD[:].rearrange("p (h c) -> p h c", c=32)
                        for half, prv in enumerate([prv0, prv1]):
                            pa = prv[:, :, 0:32].rearrange(
                                "p (h f) c -> p h f c", f=4)
                            rc4 = rec[:, 12 * half:12 * half + 12].rearrange(
                                "p (h f) -> p h f", f=4)
                            rl4 = recl[:, 12 * half:12 * half + 12].rearrange(
                                "p (h f) -> p h f", f=4)
                            ha = 3 * half
                            nc.vector.tensor_mul(
                                av[:, ha:ha + 3, :], pa[:, :, 0, :],
                                rc4[:, :, 0:1].broadcast_to([128, 3, 32]))
                            nc.vector.tensor_mul(
                                dv[:, ha:ha + 3, :], pa[:, :, 1, :],
                                rl4[:, :, 1:2].broadcast_to([128, 3, 32]))
                            nc.vector.tensor_mul(
                                av[:, 6 + ha:6 + ha + 3, :], pa[:, :, 2, :],
                                rc4[:, :, 2:3].broadcast_to([128, 3, 32]))
                            nc.vector.tensor_mul(
                                dv[:, 6 + ha:6 + ha + 3, :], pa[:, :, 3, :],
                                rl4[:, :, 3:4].broadcast_to([128, 3, 32]))
                        oc_t = opool.tile([128, 384], bf, tag="oc")
                        nc.vector.tensor_sub(oc_t[:], tA[:], tD[:])

                        # ---- transpose to feature-major ----
                        trp = ps_tr.tile([128, 512], bf, tag="trp")
                        for ch in range(4):
                            nc.tensor.transpose(
                                trp[0:96, ch * 128:(ch + 1) * 128],
                                oc_t[:, ch * 96:(ch + 1) * 96], id_t[:])
                        otT = opool.tile([96, 512], bf, tag="otT")
                        nc.any.tensor_copy(otT[:], trp[0:96, :])

                        # ---- proj + output ----
                        for st in range(2):
                            ups = ps_u.tile([128, 256], f32, tag="u")
                            for ocn in range(2):
                                for kc in range(2):
                                    mv = otT[:, st * 256 + kc * 128:
                                             st * 256 + (kc + 1) * 128]
                                    wk = pt_t[br][kc]
                                    if ocn == 0:
                                        nc.tensor.matmul(
                                            ups[:, 0:128], wk[:, 0:128], mv,
                                            start=(kc == 0), stop=(kc == 1))
                                    else:
                                        nc.tensor.matmul(
                                            ups[0:64, 128:256], wk[:, 128:192], mv,
                                            start=(kc == 0), stop=(kc == 1))
                            ou = opool.tile([128, 256], f32, tag="ou")
                            nc.any.tensor_copy(ou[:, 0:128], ups[:, 0:128])
                            nc.any.tensor_copy(ou[0:64, 128:256],
                                               ups[0:64, 128:256])
                            qd = br * 2 + st
                            w1 = g * GROUP + tb * 2
                            for w in range(2):
                                nc.sync.dma_start(
                                    out=out_d[qd, w1 + w, 0:128, :],
                                    in_=ou[:, w * 64:w * 64 + 64])
                                nc.sync.dma_start(
                                    out=out_d[qd, w1 + w, 128:192, :],
                                    in_=ou[0:64, 128 + w * 64:128 + w * 64 + 64])
    nc.finalize()
    return nc


def _get_compiled(lam_sa, lam_ca):
    key = (round(lam_sa, 9), round(lam_ca, 9))
    if key not in _COMPILED:
        _COMPILED[key] = _build_nc(lam_sa, lam_ca)
    return _COMPILED[key]


def _run(nc, in_maps):
    from concourse.bass_utils import run_bass_kernel_spmd
    res = run_bass_kernel_spmd(nc, in_maps, list(range(N_CORES)))
    return res.results


def kernel(**inputs):
    shared, zs, (lam_sa, lam_ca) = _host_prep(inputs)
    nc = _get_compiled(lam_sa, lam_ca)
    in_maps = [{"zt": zs[c], **shared} for c in range(N_CORES)]
    results = _run(nc, in_maps)
    out = np.empty((4 * NW, N, DIM), dtype=F32)
    for c in range(N_CORES):
        o = results[c]["outT"]
        w0 = c * NWC
        # quarters: 0=sa_t, 1=sa_r; ca_out is interleaved (2b -> ca_t, 2b+1 -> ca_r)
        out[w0:w0 + NWC] = o[0].transpose(0, 2, 1)
        out[NW + w0:NW + w0 + NWC] = o[1].transpose(0, 2, 1)
        out[2 * NW + 2 * w0:2 * NW + 2 * (w0 + NWC):2] = o[2].transpose(0, 2, 1)
        out[2 * NW + 2 * w0 + 1:2 * NW + 2 * (w0 + NWC):2] = o[3].transpose(0, 2, 1)
    return out

